# revision 1
# baseline (speedup 1.0000x reference)
"""DeformableConv2D (DCNv2) forward on 8 Trainium2 NeuronCores.

Data-parallel over batch: one sample per core. Per core: offset conv on the
tensor engine (fp16 operands, fp32 accumulate); sampling coordinates and
bilinear weights on the vector engine; modulated bilinear sampling via SWDGE
dma_gather of 2x2-patch rows; corner combination via broadcast multiply
(split vector/gpsimd) + accumulating PE transposes; im2col GEMM on the
tensor engine.
"""
import sys
sys.path.insert(0, "/opt/trn_rl_repo")

import numpy as np
import ml_dtypes

import concourse.bass as bass
import concourse.bacc as bacc
import concourse.mybir as mybir
import concourse.tile as tile
from concourse import library_config

F32 = mybir.dt.float32
F16 = mybir.dt.float16
I16 = mybir.dt.int16
AL = mybir.AluOpType

H = W = 64
C = 128
F = 256
K = 9
PADR = 8                 # padded-coordinate margin
HP = WP = 80             # padded image
NPIX = H * W             # 4096
NBLK = 32                # pixel blocks of 128 (2 rows each)
CONVW = 66               # conv grid width (pad 1)
CONVN = 4608             # padded conv output length (9 tiles of 512)
XCLM = 67 + CONVN + 67   # xcl with shift margins
NROWS = 2 * HP * 40      # pair-table rows = 6400
NSLOT = 18               # gathered rows per pixel = (k, yc)
NCHUNK = 72              # gather instructions (8 slots x 128 px each)

DY = np.repeat(np.arange(3) - 1, 3).astype(np.float32)   # per-tap dy
DX = np.tile(np.arange(3) - 1, 3).astype(np.float32)     # per-tap dx


def bcast(ap, shape):
    return ap.to_broadcast(list(shape))


_NC = None


def build_nc():
    nc = bacc.Bacc("TRN2", target_bir_lowering=False)
    xcl = nc.dram_tensor("xcl", [C, XCLM], F16, kind="ExternalInput")
    pairs = nc.dram_tensor("pairs", [NROWS, 512], F16, kind="ExternalInput")
    offk = nc.dram_tensor("offk", [C, K * 27], F16, kind="ExternalInput")
    offb = nc.dram_tensor("offb", [27, 1], F32, kind="ExternalInput")
    filt = nc.dram_tensor("filt", [C, K * 2 * 128], F16, kind="ExternalInput")
    eye32 = nc.dram_tensor("eye32", [128, 128], F32, kind="ExternalInput")
    eye16 = nc.dram_tensor("eye16", [128, 128], F16, kind="ExternalInput")
    # consts: Y_all [128,32], dy/dx rows [128,9] each, X_all [128,1]
    consts = nc.dram_tensor("consts", [128, 51], F32, kind="ExternalInput")
    out_d = nc.dram_tensor("out", [2, 128, NPIX], F32, kind="ExternalOutput")

    with tile.TileContext(nc) as tc:
        with (
            tc.tile_pool(name="const", bufs=1) as cpool,
            tc.tile_pool(name="bwork", bufs=1) as bpool,
            tc.tile_pool(name="dram", bufs=1, space="DRAM") as dpool,
            tc.tile_pool(name="ps2", bufs=2, space="PSUM") as ps2pool,
        ):
            nc.gpsimd.load_library(library_config.mlp)

            s_offb = cpool.tile([27, 1], F32)
            nc.sync.dma_start(out=s_offb[:], in_=offb[:])
            s_filt = cpool.tile([C, K * 2 * 128], F16)
            nc.sync.dma_start(out=s_filt[:], in_=filt[:])
            s_eye32 = cpool.tile([128, 128], F32)
            nc.sync.dma_start(out=s_eye32[:], in_=eye32[:])
            s_eye16 = cpool.tile([128, 128], F16)
            nc.sync.dma_start(out=s_eye16[:], in_=eye16[:])
            s_const = cpool.tile([128, 51], F32)
            nc.sync.dma_start(out=s_const[:], in_=consts[:])
            y_all = s_const[:, 0:32]          # [128, 32]
            dy_t = s_const[:, 32:41]          # [128, 9]
            dx_t = s_const[:, 41:50]
            x_all = s_const[:, 50:51]         # [128, 1]

            # survivors of the conv/stage-B phase
            w16 = cpool.tile([128, 32, 36], F16)
            idxw = cpool.tile([128, 2304], I16)     # wrapped gather indices

            with (
                tc.tile_pool(name="conv", bufs=1) as vpool,
                tc.tile_pool(name="ps", bufs=2, space="PSUM") as pspool,
            ):
                # ---- loads ----
                s_xcl = vpool.tile([C, XCLM], F16)
                nc.sync.dma_start(out=s_xcl[:], in_=xcl[:])
                s_offk = vpool.tile([C, K * 27], F16)
                nc.sync.dma_start(out=s_offk[:], in_=offk[:])

                # ---- P1: offset conv (row-pair tiles) + P2 transposes ----
                wi_c = vpool.tile([27, 32, 128], F32)
                wiT = bpool.tile([128, 32, 27], F32)
                for b in range(NBLK):
                    q0 = (2 * b + 1) * CONVW
                    ps = pspool.tile([27, 132], F32, tag="convps")
                    for t in range(K):
                        d = int(DY[t]) * CONVW + int(DX[t])
                        nc.tensor.matmul(
                            out=ps[:],
                            lhsT=s_offk[:, t * 27:(t + 1) * 27],
                            rhs=s_xcl[:, 67 + q0 + d: 67 + q0 + 132 + d],
                            start=(t == 0), stop=(t == K - 1),
                        )
                    nc.vector.tensor_scalar(
                        out=wi_c[:, b, :].rearrange("p (r x) -> p r x", x=64),
                        in0=ps[:].rearrange("p (r x) -> p r x", x=CONVW)[:, :, 1:65],
                        scalar1=s_offb[:, 0:1], scalar2=None, op0=AL.add)
                    pt = pspool.tile([128, 27], F32, tag="wiTps")
                    nc.tensor.transpose(
                        out=pt[:], in_=wi_c[:, b, :], identity=s_eye32[:27, :27])
                    nc.vector.tensor_copy(out=wiT[:, b, :], in_=pt[:])

            # ---- P3: stage B ----
            o1 = wiT[:, :, 0:9]
            o2 = wiT[:, :, 9:18]
            mm = wiT[:, :, 18:27]
            S = [128, 32, 9]

            sigm = bpool.tile(S, F32)
            nc.scalar.activation(sigm[:], mm, mybir.ActivationFunctionType.Sigmoid)

            py = bpool.tile(S, F32, tag="py")
            nc.vector.tensor_tensor(
                out=py[:], in0=o1, in1=bcast(y_all, S),
                op=AL.add)
            nc.vector.tensor_tensor(
                out=py[:], in0=py[:], in1=bcast(dy_t.rearrange("p (o k) -> p o k", o=1), S),
                op=AL.add)
            nc.vector.tensor_scalar(out=py[:], in0=py[:], scalar1=8.0, scalar2=2.0,
                                    op0=AL.add, op1=AL.max)
            nc.vector.tensor_scalar(out=py[:], in0=py[:], scalar1=77.0, scalar2=None, op0=AL.min)
            y0p = bpool.tile(S, F32, tag="y0p")
            nc.vector.tensor_scalar(out=y0p[:], in0=py[:], scalar1=-0.5,
                                    scalar2=8388608.0, op0=AL.add, op1=AL.add)
            nc.vector.tensor_scalar(out=y0p[:], in0=y0p[:], scalar1=-8388608.0,
                                    scalar2=None, op0=AL.add)
            fy = bpool.tile(S, F32, tag="fy")
            nc.vector.tensor_tensor(out=fy[:], in0=py[:], in1=y0p[:], op=AL.subtract)
            wy0 = bpool.tile(S, F32, tag="wy0")
            nc.vector.tensor_scalar(out=wy0[:], in0=fy[:], scalar1=-1.0, scalar2=1.0,
                                    op0=AL.mult, op1=AL.add)

            px = bpool.tile(S, F32, tag="px")
            nc.vector.tensor_tensor(
                out=px[:], in0=o2,
                in1=bcast(x_all, S), op=AL.add)
            nc.vector.tensor_tensor(
                out=px[:], in0=px[:], in1=bcast(dx_t.rearrange("p (o k) -> p o k", o=1), S),
                op=AL.add)
            nc.vector.tensor_scalar(out=px[:], in0=px[:], scalar1=8.0, scalar2=2.0,
                                    op0=AL.add, op1=AL.max)
            nc.vector.tensor_scalar(out=px[:], in0=px[:], scalar1=77.0, scalar2=None, op0=AL.min)
            x0p = bpool.tile(S, F32, tag="x0p")
            nc.vector.tensor_scalar(out=x0p[:], in0=px[:], scalar1=-0.5,
                                    scalar2=8388608.0, op0=AL.add, op1=AL.add)
            nc.vector.tensor_scalar(out=x0p[:], in0=x0p[:], scalar1=-8388608.0,
                                    scalar2=None, op0=AL.add)
            fx = bpool.tile(S, F32, tag="fx")
            nc.vector.tensor_tensor(out=fx[:], in0=px[:], in1=x0p[:], op=AL.subtract)
            wx0 = bpool.tile(S, F32, tag="wx0")
            nc.vector.tensor_scalar(out=wx0[:], in0=fx[:], scalar1=-1.0, scalar2=1.0,
                                    op0=AL.mult, op1=AL.add)

            qx = bpool.tile(S, F32, tag="qx")
            nc.vector.tensor_scalar(out=qx[:], in0=x0p[:], scalar1=0.5,
                                    scalar2=-0.25, op0=AL.mult, op1=AL.add)
            nc.vector.tensor_scalar(out=qx[:], in0=qx[:], scalar1=8388608.0,
                                    scalar2=-8388608.0, op0=AL.add, op1=AL.add)
            parx = bpool.tile(S, F32, tag="parx")
            nc.vector.scalar_tensor_tensor(
                out=parx[:], in0=qx[:], scalar=-2.0, in1=x0p[:],
                op0=AL.mult, op1=AL.add)
            qy = bpool.tile(S, F32, tag="qy")
            nc.vector.tensor_scalar(out=qy[:], in0=y0p[:], scalar1=0.5,
                                    scalar2=-0.25, op0=AL.mult, op1=AL.add)
            nc.vector.tensor_scalar(out=qy[:], in0=qy[:], scalar1=8388608.0,
                                    scalar2=-8388608.0, op0=AL.add, op1=AL.add)
            pary = bpool.tile(S, F32, tag="pary")
            nc.vector.scalar_tensor_tensor(
                out=pary[:], in0=qy[:], scalar=-2.0, in1=y0p[:],
                op0=AL.mult, op1=AL.add)
            base = bpool.tile(S, F32, tag="base")
            nc.vector.scalar_tensor_tensor(
                out=base[:], in0=qy[:], scalar=40.0, in1=qx[:],
                op0=AL.mult, op1=AL.add)
            nc.vector.scalar_tensor_tensor(
                out=base[:], in0=parx[:], scalar=1600.0, in1=base[:],
                op0=AL.mult, op1=AL.add)
            nc.vector.scalar_tensor_tensor(
                out=base[:], in0=pary[:], scalar=3200.0, in1=base[:],
                op0=AL.mult, op1=AL.add)

            idx_i16 = bpool.tile([128, 32, 9], I16)
            nc.vector.tensor_copy(out=idx_i16[:], in_=base[:])

            # wrapped-index staging: DRAM round trip
            idx_dram = dpool.tile([128, 288], I16)
            nc.sync.dma_start(out=idx_dram[:],
                              in_=idx_i16[:].rearrange("p b k -> p (b k)"))
            # wrapped[q + 16rep, (g, pg)] = flatidx[(pg*16+q)*288 + g]
            for g in range(8):
                nc.sync.dma_start(
                    out=idxw[16 * g:16 * (g + 1), :],
                    in_=idx_dram[:].rearrange(
                        "(pg q) g -> q g pg", pg=8))

            # weights W [128, 32, 9, 2, 2]  (k, yc, u)
            a0 = bpool.tile(S, F32, tag="a0")
            nc.vector.tensor_tensor(out=a0[:], in0=wy0[:], in1=sigm[:], op=AL.mult)
            a1 = bpool.tile(S, F32, tag="a1")
            nc.vector.tensor_tensor(out=a1[:], in0=fy[:], in1=sigm[:], op=AL.mult)
            w_f32 = bpool.tile([128, 32, 9, 2, 2], F32)
            nc.vector.tensor_tensor(out=w_f32[:, :, :, 0, 0], in0=a0[:], in1=wx0[:],
                                    op=AL.mult)
            nc.vector.tensor_tensor(out=w_f32[:, :, :, 0, 1], in0=a0[:], in1=fx[:],
                                    op=AL.mult)
            nc.vector.tensor_tensor(out=w_f32[:, :, :, 1, 0], in0=a1[:], in1=wx0[:],
                                    op=AL.mult)
            nc.vector.tensor_tensor(out=w_f32[:, :, :, 1, 1], in0=a1[:], in1=fx[:],
                                    op=AL.mult)
            nc.vector.tensor_copy(
                out=w16[:], in_=w_f32[:].rearrange("p b k y u -> p b (k y u)"))

            # ---- P4 ----
            with (
                tc.tile_pool(name="sgpool", bufs=2) as sgpool,
                tc.tile_pool(name="blkpool", bufs=2) as blkpool,
                tc.tile_pool(name="ps3", bufs=4, space="PSUM") as ps3pool,
            ):
                for sg in range(8):
                    dst = sgpool.tile([128, 36, 512], F16, tag="dst")
                    g0 = sg * 36
                    for cl in range(5):
                        lo = 8 * cl
                        ns = min(8, 36 - lo)
                        nc.gpsimd.dma_gather(
                            dst[:, lo:lo + ns, :], pairs[:],
                            idxw[:, (g0 + lo) * 8:(g0 + lo + ns) * 8],
                            ns * 128, ns * 128, 512)
                    cols = sgpool.tile([128, K, 512], F16, tag="cols")
                    for bi in range(4):
                        b = sg * 4 + bi
                        gw = blkpool.tile([128, 36, 128], F16, tag="gw")
                        dsrc = dst[:, 9 * bi:9 * (bi + 1), :].rearrange(
                            "p s e -> p (s e)").rearrange(
                            "p (j c) -> p j c", c=128)
                        nc.vector.tensor_tensor(
                            out=gw[:, 0:24, :], in0=dsrc[:, 0:24, :],
                            in1=bcast(w16[:, b, 0:24], [128, 24, 128]),
                            op=AL.mult)
                        nc.gpsimd.tensor_tensor(
                            out=gw[:, 24:36, :], in0=dsrc[:, 24:36, :],
                            in1=bcast(w16[:, b, 24:36], [128, 12, 128]),
                            op=AL.mult)
                        for k in range(K):
                            pc = ps3pool.tile([128, 128], F32, tag="ctps")
                            for j in range(4):
                                nc.tensor.matmul(
                                    out=pc[:], lhsT=gw[:, 4 * k + j, :],
                                    rhs=s_eye16[:], start=(j == 0), stop=(j == 3))
                            if k % 2 == 0:
                                nc.scalar.copy(
                                    out=cols[:, k, bi * 128:(bi + 1) * 128],
                                    in_=pc[:])
                            else:
                                nc.vector.tensor_copy(
                                    out=cols[:, k, bi * 128:(bi + 1) * 128],
                                    in_=pc[:])
                    for fc in range(2):
                        po = ps2pool.tile([128, 512], F32, tag="outps")
                        for k in range(K):
                            nc.tensor.matmul(
                                out=po[:],
                                lhsT=s_filt[:, (k * 2 + fc) * 128:
                                            (k * 2 + fc + 1) * 128],
                                rhs=cols[:, k, :],
                                start=(k == 0), stop=(k == K - 1))
                        osb = blkpool.tile([128, 512], F32, tag="osb")
                        if fc == 0:
                            nc.scalar.copy(out=osb[:], in_=po[:])
                        else:
                            nc.vector.tensor_copy(out=osb[:], in_=po[:])
                        nc.sync.dma_start(
                            out=out_d[fc, :, sg * 512:(sg + 1) * 512], in_=osb[:])
    nc.compile()
    return nc


def host_inputs(x, offset_kernel, offset_bias, filt_w):
    """Per-sample input maps. x [8,64,64,128] f32 etc (numpy)."""
    offk = np.ascontiguousarray(
        offset_kernel.reshape(K, C, 27).transpose(1, 0, 2).reshape(C, K * 27)
    ).astype(np.float16)
    offb = offset_bias.reshape(27, 1).astype(np.float32)
    filt_re = np.ascontiguousarray(
        filt_w.reshape(K, C, 2, 128).transpose(1, 0, 2, 3).reshape(C, K * 2 * 128)
    ).astype(np.float16)
    eye32 = np.eye(128, dtype=np.float32)
    eye16 = np.eye(128).astype(np.float16)
    consts = np.zeros((128, 51), np.float32)
    p = np.arange(128)
    yoff = p // 64
    consts[:, 0:32] = 2 * np.arange(32)[None, :] + yoff[:, None]
    consts[:, 32:41] = DY[None, :]
    consts[:, 41:50] = DX[None, :]
    consts[:, 50] = p % 64

    maps = []
    for b in range(x.shape[0]):
        xp = np.zeros((HP + 2, WP + 2, C), np.float32)
        xp[PADR:PADR + H, PADR:PADR + W] = x[b]
        quad = np.zeros((2, 2, 40, 40, 2, 2, C), np.float32)
        for pY in range(2):
            for pX in range(2):
                for uy in range(2):
                    for ux in range(2):
                        quad[pY, pX, :, :, uy, ux] = \
                            xp[pY + uy:pY + uy + 80:2, pX + ux:pX + ux + 80:2]
        prs = quad.reshape(NROWS, 4 * C).astype(np.float16)

        x1 = np.zeros((CONVW, CONVW, C), np.float32)
        x1[1:65, 1:65] = x[b]
        xcl = np.zeros((C, XCLM), np.float16)
        xcl[:, 67:67 + 4356] = x1.reshape(CONVW * CONVW, C).T.astype(np.float16)
        maps.append({
            "xcl": xcl, "pairs": prs, "offk": offk, "offb": offb,
            "filt": filt_re, "eye32": eye32, "eye16": eye16, "consts": consts,
        })
    return maps


def host_output(res_list):
    outs = []
    for r in res_list:
        o = r["out"].reshape(256, NPIX)
        outs.append(np.ascontiguousarray(o.T).reshape(H, W, F))
    return np.stack(outs)


def _get_nc():
    global _NC
    if _NC is None:
        _NC = build_nc()
    return _NC


def kernel(inputs, offset_kernel, offset_bias, filt):
    from concourse.bass_utils import run_bass_kernel_spmd
    x = np.asarray(inputs, dtype=np.float32)
    maps = host_inputs(x, np.asarray(offset_kernel, np.float32),
                       np.asarray(offset_bias, np.float32),
                       np.asarray(filt, np.float32))
    nc = _get_nc()
    res = run_bass_kernel_spmd(nc, maps, core_ids=list(range(8)))
    return host_output(res.results).astype(np.float32)



# revision 9
# speedup vs baseline: 1.8446x; 1.8446x over previous
"""DeformableConv2D (DCNv2) forward on 8 Trainium2 NeuronCores.

Data-parallel over batch: one sample per core. Per core: offset conv on the
tensor engine (fp16 operands, fp32 accumulate); sampling coordinates and
bilinear weights on the vector engine (sigmoid via odd polynomial, no act
tables); modulated bilinear sampling via SWDGE dma_gather of 2x2-patch rows;
corner combination via packed-fp16 broadcast multiply (DVE 2x mode) +
accumulating PE transposes; im2col GEMM on the tensor engine. The front end
is split into three chunks (4/12/16 blocks) pipelined against the gather
stream, and gather descriptor generation is kept ahead of per-block compute
on the Pool queue.
"""
import sys
sys.path.insert(0, "/opt/trn_rl_repo")

import numpy as np
import ml_dtypes

import concourse.bass as bass
import concourse.bacc as bacc
import concourse.mybir as mybir
import concourse.tile as tile
from concourse import library_config

F32 = mybir.dt.float32
F16 = mybir.dt.float16
I16 = mybir.dt.int16
AL = mybir.AluOpType
AF = mybir.ActivationFunctionType

H = W = 64
C = 128
F = 256
K = 9
PADR = 8                 # padded-coordinate margin
HP = WP = 80             # padded image
NPIX = H * W             # 4096
NBLK = 32                # pixel blocks of 128 (2 rows each)
CONVW = 66               # conv grid width (pad 1)
XCLM = 67 + 9 * 512 + 67  # xcl with shift margins
NROWS = 2 * HP * 40      # pair-table rows = 6400
CHUNKS = [(0, 4), (4, 16), (16, 32)]   # front-end block chunks

# sigmoid(x) ~= 0.5 + x*(C1 + C3 z + C5 z^2 + C7 z^3), z = x^2, |x| <= 2.75
SB = 2.75
SC1, SC3, SC5, SC7 = 0.24955315, -0.019879351, 1.5030454e-3, -5.8584555e-5

DY = np.repeat(np.arange(3) - 1, 3).astype(np.float32)   # per-tap dy
DX = np.tile(np.arange(3) - 1, 3).astype(np.float32)     # per-tap dx


def bcast(ap, shape):
    return ap.to_broadcast(list(shape))


_NC = None


def build_nc():
    nc = bacc.Bacc("TRN2", target_bir_lowering=False)
    xcl = nc.dram_tensor("xcl", [C, XCLM], F16, kind="ExternalInput")
    pairs = nc.dram_tensor("pairs", [NROWS, 512], F16, kind="ExternalInput")
    offk = nc.dram_tensor("offk", [C, K * 27], F16, kind="ExternalInput")
    offb = nc.dram_tensor("offb", [27, 1], F32, kind="ExternalInput")
    filt = nc.dram_tensor("filt", [C, K * 2 * 128], F16, kind="ExternalInput")
    eye16 = nc.dram_tensor("eye16", [128, 128], F16, kind="ExternalInput")
    # consts: Y_all [128,32], dy/dx rows [128,9] each, X_all [128,1]
    consts = nc.dram_tensor("consts", [128, 51], F32, kind="ExternalInput")
    out_d = nc.dram_tensor("out", [2, 128, NPIX], F16, kind="ExternalOutput")

    with tile.TileContext(nc) as tc:
        with (
            tc.tile_pool(name="const", bufs=1) as cpool,
            tc.tile_pool(name="front", bufs=1) as fpool,
            tc.tile_pool(name="dram", bufs=1, space="DRAM") as dpool,
            tc.tile_pool(name="convps", bufs=1, space="PSUM") as convpool,
            tc.tile_pool(name="ptps", bufs=1, space="PSUM") as ptpool,
            tc.tile_pool(name="pc0", bufs=2, space="PSUM") as pc0pool,
            tc.tile_pool(name="pc1", bufs=1, space="PSUM") as pc1pool,
            tc.tile_pool(name="pc2", bufs=1, space="PSUM") as pc2pool,
            tc.tile_pool(name="po", bufs=2, space="PSUM") as popool,
            tc.tile_pool(name="sg", bufs=2) as sgpool,
            tc.tile_pool(name="blk", bufs=2) as blkpool,
        ):
            nc.gpsimd.load_library(library_config.mlp)

            s_offb = cpool.tile([27, 1], F32)
            nc.sync.dma_start(out=s_offb[:], in_=offb[:])
            s_filt = cpool.tile([C, K * 2 * 128], F16)
            nc.sync.dma_start(out=s_filt[:], in_=filt[:])
            s_eye16 = cpool.tile([128, 128], F16)
            nc.sync.dma_start(out=s_eye16[:], in_=eye16[:])
            s_const = cpool.tile([128, 51], F32)
            nc.sync.dma_start(out=s_const[:], in_=consts[:])
            s_xcl = cpool.tile([C, XCLM], F16)
            nc.sync.dma_start(out=s_xcl[:], in_=xcl[:])
            s_offk = cpool.tile([C, K * 27], F16)
            nc.sync.dma_start(out=s_offk[:], in_=offk[:])
            s_eye27 = cpool.tile([27, 27], F32)
            nc.vector.tensor_copy(out=s_eye27[:], in_=s_eye16[:27, :27])
            y_all = s_const[:, 0:32]          # [128, 32]
            dy_t = s_const[:, 32:41]          # [128, 9]
            dx_t = s_const[:, 41:50]
            x_all = s_const[:, 50:51]         # [128, 1]

            w16_c = [None] * 3                # per-chunk weights [128,n,36] f16
            idxw_c = [None] * 3               # per-chunk wrapped indices
            dst_t = {}                        # per-sg gather destinations

            def front_chunk(ci):
                """Offset conv + coordinate/weight math for blocks [lo, hi);
                fills w16_c[ci], idxw_c[ci]."""
                lo, hi = CHUNKS[ci]
                n = hi - lo
                wi_c = fpool.tile([27, n, 128], F32, tag=f"wic{ci}")
                wiT = fpool.tile([128, n, 27], F32, tag=f"wiT{ci}")
                for bi in range(0, n, 2):
                    b = lo + bi
                    q0 = (2 * b + 1) * CONVW
                    ps = convpool.tile([27, 264], F32, tag="convps")
                    for t in range(K):
                        d = int(DY[t]) * CONVW + int(DX[t])
                        nc.tensor.matmul(
                            out=ps[:],
                            lhsT=s_offk[:, t * 27:(t + 1) * 27],
                            rhs=s_xcl[:, 67 + q0 + d: 67 + q0 + 264 + d],
                            start=(t == 0), stop=(t == K - 1),
                        )
                    # bias + f32 copy on the activation engine
                    nc.scalar.add(
                        out=wi_c[:, bi:bi + 2, :].rearrange(
                            "p b (r x) -> p (b r) x", x=64),
                        in_=ps[:].rearrange("p (r x) -> p r x", x=CONVW)[:, :, 1:65],
                        add=s_offb[:, 0:1])
                    for u in range(2):
                        pt = ptpool.tile([128, 27], F32, tag="pt")
                        nc.tensor.transpose(
                            out=pt[:], in_=wi_c[:, bi + u, :], identity=s_eye27[:])
                        nc.scalar.copy(out=wiT[:, bi + u, :], in_=pt[:])

                # ---- stage B on [128, n, 9] tiles ----
                o1 = wiT[:, :, 0:9]
                o2 = wiT[:, :, 9:18]
                mm = wiT[:, :, 18:27]
                S = [128, n, 9]
                yh = y_all[:, lo:hi]

                def ftile(tag):
                    return fpool.tile(S, F32, tag=f"{tag}{ci}",
                                      name=f"{tag}{ci}")

                # sigmoid via odd polynomial on DVE (no act-table loads)
                xc = ftile("xc")
                nc.vector.tensor_scalar(out=xc[:], in0=mm, scalar1=-SB,
                                        scalar2=SB, op0=AL.max, op1=AL.min)
                zz = ftile("zz")
                nc.vector.tensor_tensor(out=zz[:], in0=xc[:], in1=xc[:],
                                        op=AL.mult)
                sigm = ftile("sigm")
                nc.vector.tensor_scalar(out=sigm[:], in0=zz[:], scalar1=SC7,
                                        scalar2=SC5, op0=AL.mult, op1=AL.add)
                nc.vector.tensor_tensor(out=sigm[:], in0=sigm[:], in1=zz[:],
                                        op=AL.mult)
                nc.vector.tensor_scalar(out=sigm[:], in0=sigm[:], scalar1=SC3,
                                        scalar2=None, op0=AL.add)
                nc.vector.tensor_tensor(out=sigm[:], in0=sigm[:], in1=zz[:],
                                        op=AL.mult)
                nc.vector.tensor_scalar(out=sigm[:], in0=sigm[:], scalar1=SC1,
                                        scalar2=None, op0=AL.add)
                nc.vector.tensor_tensor(out=sigm[:], in0=sigm[:], in1=xc[:],
                                        op=AL.mult)
                nc.vector.tensor_scalar(out=sigm[:], in0=sigm[:], scalar1=0.5,
                                        scalar2=None, op0=AL.add)

                py = ftile("py")
                nc.vector.tensor_tensor(out=py[:], in0=o1, in1=bcast(yh, S),
                                        op=AL.add)
                nc.vector.tensor_tensor(
                    out=py[:], in0=py[:],
                    in1=bcast(dy_t.rearrange("p (o k) -> p o k", o=1), S),
                    op=AL.add)
                nc.vector.tensor_scalar(out=py[:], in0=py[:], scalar1=8.0,
                                        scalar2=2.0, op0=AL.add, op1=AL.max)
                nc.vector.tensor_scalar(out=py[:], in0=py[:], scalar1=77.0,
                                        scalar2=None, op0=AL.min)
                y0p = ftile("y0p")
                nc.vector.tensor_scalar(out=y0p[:], in0=py[:], scalar1=-0.5,
                                        scalar2=8388608.0, op0=AL.add, op1=AL.add)
                nc.vector.tensor_scalar(out=y0p[:], in0=y0p[:], scalar1=-8388608.0,
                                        scalar2=None, op0=AL.add)
                fy = ftile("fy")
                nc.vector.tensor_tensor(out=fy[:], in0=py[:], in1=y0p[:],
                                        op=AL.subtract)
                wy0 = ftile("wy0")
                nc.vector.tensor_scalar(out=wy0[:], in0=fy[:], scalar1=-1.0,
                                        scalar2=1.0, op0=AL.mult, op1=AL.add)

                px = ftile("px")
                nc.vector.tensor_tensor(out=px[:], in0=o2, in1=bcast(x_all, S),
                                        op=AL.add)
                nc.vector.tensor_tensor(
                    out=px[:], in0=px[:],
                    in1=bcast(dx_t.rearrange("p (o k) -> p o k", o=1), S),
                    op=AL.add)
                nc.vector.tensor_scalar(out=px[:], in0=px[:], scalar1=8.0,
                                        scalar2=2.0, op0=AL.add, op1=AL.max)
                nc.vector.tensor_scalar(out=px[:], in0=px[:], scalar1=77.0,
                                        scalar2=None, op0=AL.min)
                x0p = ftile("x0p")
                nc.vector.tensor_scalar(out=x0p[:], in0=px[:], scalar1=-0.5,
                                        scalar2=8388608.0, op0=AL.add, op1=AL.add)
                nc.vector.tensor_scalar(out=x0p[:], in0=x0p[:], scalar1=-8388608.0,
                                        scalar2=None, op0=AL.add)
                fx = ftile("fx")
                nc.vector.tensor_tensor(out=fx[:], in0=px[:], in1=x0p[:],
                                        op=AL.subtract)
                wx0 = ftile("wx0")
                nc.vector.tensor_scalar(out=wx0[:], in0=fx[:], scalar1=-1.0,
                                        scalar2=1.0, op0=AL.mult, op1=AL.add)

                qx = ftile("qx")
                nc.vector.tensor_scalar(out=qx[:], in0=x0p[:], scalar1=0.5,
                                        scalar2=-0.25, op0=AL.mult, op1=AL.add)
                nc.vector.tensor_scalar(out=qx[:], in0=qx[:], scalar1=8388608.0,
                                        scalar2=-8388608.0, op0=AL.add, op1=AL.add)
                parx = ftile("parx")
                nc.vector.scalar_tensor_tensor(
                    out=parx[:], in0=qx[:], scalar=-2.0, in1=x0p[:],
                    op0=AL.mult, op1=AL.add)
                qy = ftile("qy")
                nc.vector.tensor_scalar(out=qy[:], in0=y0p[:], scalar1=0.5,
                                        scalar2=-0.25, op0=AL.mult, op1=AL.add)
                nc.vector.tensor_scalar(out=qy[:], in0=qy[:], scalar1=8388608.0,
                                        scalar2=-8388608.0, op0=AL.add, op1=AL.add)
                pary = ftile("pary")
                nc.vector.scalar_tensor_tensor(
                    out=pary[:], in0=qy[:], scalar=-2.0, in1=y0p[:],
                    op0=AL.mult, op1=AL.add)
                base = ftile("base")
                nc.vector.scalar_tensor_tensor(
                    out=base[:], in0=qy[:], scalar=40.0, in1=qx[:],
                    op0=AL.mult, op1=AL.add)
                nc.vector.scalar_tensor_tensor(
                    out=base[:], in0=parx[:], scalar=1600.0, in1=base[:],
                    op0=AL.mult, op1=AL.add)
                nc.vector.scalar_tensor_tensor(
                    out=base[:], in0=pary[:], scalar=3200.0, in1=base[:],
                    op0=AL.mult, op1=AL.add)

                idx_i16 = fpool.tile([128, n * 9], I16, tag=f"idxi{ci}")
                nc.vector.tensor_copy(
                    out=idx_i16[:].rearrange("p (b k) -> p b k", k=9), in_=base[:])

                # wrapped-index staging: DRAM roundtrip with 2-row-sized
                # descriptors, then an on-chip (pg, g) -> (g, pg) interleave.
                idx_dram = dpool.tile([128, n * 9], I16, tag=f"idxd{ci}")
                nc.sync.dma_start(out=idx_dram[:], in_=idx_i16[:])
                idxw_tmp = fpool.tile([128, n * 72], I16, tag=f"idxt{ci}")
                src = idx_dram[:].rearrange("(pg pp) g -> pp pg g", pg=8)
                for r in range(8):
                    nc.sync.dma_start(
                        out=idxw_tmp[16 * r:16 * (r + 1), :].rearrange(
                            "pp (pg g) -> pp pg g", pg=8),
                        in_=src)
                idxw = fpool.tile([128, n * 72], I16, tag=f"idxw{ci}")
                nc.vector.tensor_copy(
                    out=idxw[:].rearrange("p (g pg) -> p g pg", pg=8),
                    in_=idxw_tmp[:].rearrange("p (pg g) -> p g pg", pg=8))
                idxw_c[ci] = idxw

                # weights W [128, n, 9, 2, 2]  (k, yc, xc)
                a0 = ftile("a0")
                nc.vector.tensor_tensor(out=a0[:], in0=wy0[:], in1=sigm[:],
                                        op=AL.mult)
                a1 = ftile("a1")
                nc.vector.tensor_tensor(out=a1[:], in0=fy[:], in1=sigm[:],
                                        op=AL.mult)
                w_f32 = fpool.tile([128, n, 9, 2, 2], F32, tag=f"wf{ci}")
                nc.vector.tensor_tensor(out=w_f32[:, :, :, 0, 0], in0=a0[:],
                                        in1=wx0[:], op=AL.mult)
                nc.vector.tensor_tensor(out=w_f32[:, :, :, 0, 1], in0=a0[:],
                                        in1=fx[:], op=AL.mult)
                nc.vector.tensor_tensor(out=w_f32[:, :, :, 1, 0], in0=a1[:],
                                        in1=wx0[:], op=AL.mult)
                nc.vector.tensor_tensor(out=w_f32[:, :, :, 1, 1], in0=a1[:],
                                        in1=fx[:], op=AL.mult)
                w16 = fpool.tile([128, n, 36], F16, tag=f"w16{ci}")
                nc.vector.tensor_copy(
                    out=w16[:], in_=w_f32[:].rearrange("p b k y u -> p b (k y u)"))
                w16_c[ci] = w16

            def chunk_of(sg):
                b0 = 4 * sg
                for ci, (lo, hi) in enumerate(CHUNKS):
                    if lo <= b0 < hi:
                        return ci, lo
                raise AssertionError

            def emit_gathers(sg):
                ci, lo = chunk_of(sg)
                idxw = idxw_c[ci]
                g0 = (4 * sg - lo) * 9           # slot base within chunk
                dst = sgpool.tile([128, 36, 512], F16, tag="dst")
                for cl in range(5):
                    clo = 8 * cl
                    ns = min(8, 36 - clo)
                    nc.gpsimd.dma_gather(
                        dst[:, clo:clo + ns, :], pairs[:],
                        idxw[:, (g0 + clo) * 8:(g0 + clo + ns) * 8],
                        ns * 128, ns * 128, 512)
                dst_t[sg] = dst

            def compute_sg(sg):
                ci, lo = chunk_of(sg)
                w16 = w16_c[ci]
                dst = dst_t.pop(sg)
                cols = sgpool.tile([128, K, 512], F16, tag="cols")
                for bi in range(4):
                    bh = 4 * sg - lo + bi        # block within chunk
                    w_e8 = blkpool.tile([128, 36, 8], F16, tag="we8")
                    nc.gpsimd.tensor_copy(
                        out=w_e8[:],
                        in_=bcast(w16[:, bh, :].rearrange("p (j o) -> p j o", o=1),
                                  [128, 36, 8]))
                    gw = blkpool.tile([128, 36, 128], F16, tag="gw")
                    dsrc = dst[:, 9 * bi:9 * (bi + 1), :].rearrange(
                        "p s e -> p (s e)").rearrange(
                        "p (j r q) -> p j r q", r=16, q=8)
                    gwv = gw[:].rearrange("p j (r q) -> p j r q", q=8)
                    w_in = bcast(w_e8[:].rearrange("p j (o q) -> p j o q", o=1),
                                 [128, 36, 16, 8])
                    # packed fp16 operands -> DVE 2x mode
                    nc.vector.tensor_tensor(out=gwv[:], in0=dsrc[:], in1=w_in[:],
                                            op=AL.mult)
                    # accumulating transposes: 4 taps per PSUM bank
                    for kg, pool, nk in ((0, pc0pool, 4), (1, pc1pool, 4),
                                         (2, pc2pool, 1)):
                        pcb = pool.tile([128, nk * 128], F32, tag=f"pc{kg}")
                        for kq in range(nk):
                            k = 4 * kg + kq
                            for j in range(4):
                                nc.tensor.matmul(
                                    out=pcb[:, kq * 128:(kq + 1) * 128],
                                    lhsT=gw[:, 4 * k + j, :],
                                    rhs=s_eye16[:], start=(j == 0), stop=(j == 3))
                        nc.scalar.copy(
                            out=cols[:, 4 * kg:4 * kg + nk,
                                     bi * 128:(bi + 1) * 128],
                            in_=pcb[:].rearrange("p (k c) -> p k c", c=128))
                for fc in range(2):
                    po = popool.tile([128, 512], F32, tag="po")
                    for k in range(K):
                        nc.tensor.matmul(
                            out=po[:],
                            lhsT=s_filt[:, (k * 2 + fc) * 128:
                                        (k * 2 + fc + 1) * 128],
                            rhs=cols[:, k, :],
                            start=(k == 0), stop=(k == K - 1))
                    osb = blkpool.tile([128, 512], F16, tag="osb")
                    nc.scalar.copy(out=osb[:], in_=po[:])
                    nc.sync.dma_start(
                        out=out_d[fc, :, sg * 512:(sg + 1) * 512], in_=osb[:])

            front_chunk(0)
            emit_gathers(0)
            front_chunk(1)
            emit_gathers(1)
            compute_sg(0)
            emit_gathers(2)
            compute_sg(1)
            front_chunk(2)
            emit_gathers(3)
            compute_sg(2)
            for sg in range(3, 8):
                if sg + 1 < 8:
                    emit_gathers(sg + 1)
                compute_sg(sg)
    nc.compile()
    return nc


def host_inputs(x, offset_kernel, offset_bias, filt_w):
    """Per-sample input maps. x [8,64,64,128] f32 etc (numpy)."""
    offk = np.ascontiguousarray(
        offset_kernel.reshape(K, C, 27).transpose(1, 0, 2).reshape(C, K * 27)
    ).astype(np.float16)
    offb = offset_bias.reshape(27, 1).astype(np.float32)
    filt_re = np.ascontiguousarray(
        filt_w.reshape(K, C, 2, 128).transpose(1, 0, 2, 3).reshape(C, K * 2 * 128)
    ).astype(np.float16)
    eye16 = np.eye(128).astype(np.float16)
    consts = np.zeros((128, 51), np.float32)
    p = np.arange(128)
    yoff = p // 64
    consts[:, 0:32] = 2 * np.arange(32)[None, :] + yoff[:, None]
    consts[:, 32:41] = DY[None, :]
    consts[:, 41:50] = DX[None, :]
    consts[:, 50] = p % 64

    maps = []
    for b in range(x.shape[0]):
        xp = np.zeros((HP + 2, WP + 2, C), np.float32)
        xp[PADR:PADR + H, PADR:PADR + W] = x[b]
        quad = np.zeros((2, 2, 40, 40, 2, 2, C), np.float32)
        for pY in range(2):
            for pX in range(2):
                for uy in range(2):
                    for ux in range(2):
                        quad[pY, pX, :, :, uy, ux] = \
                            xp[pY + uy:pY + uy + 80:2, pX + ux:pX + ux + 80:2]
        prs = quad.reshape(NROWS, 4 * C).astype(np.float16)

        x1 = np.zeros((CONVW, CONVW, C), np.float32)
        x1[1:65, 1:65] = x[b]
        xcl = np.zeros((C, XCLM), np.float16)
        xcl[:, 67:67 + 4356] = x1.reshape(CONVW * CONVW, C).T.astype(np.float16)
        maps.append({
            "xcl": xcl, "pairs": prs, "offk": offk, "offb": offb,
            "filt": filt_re, "eye16": eye16, "consts": consts,
        })
    return maps


def host_output(res_list):
    outs = []
    for r in res_list:
        o = r["out"].astype(np.float32).reshape(256, NPIX)
        outs.append(np.ascontiguousarray(o.T).reshape(H, W, F))
    return np.stack(outs)


def _get_nc():
    global _NC
    if _NC is None:
        _NC = build_nc()
    return _NC


def kernel(inputs, offset_kernel, offset_bias, filt):
    from concourse.bass_utils import run_bass_kernel_spmd
    x = np.asarray(inputs, dtype=np.float32)
    maps = host_inputs(x, np.asarray(offset_kernel, np.float32),
                       np.asarray(offset_bias, np.float32),
                       np.asarray(filt, np.float32))
    nc = _get_nc()
    res = run_bass_kernel_spmd(nc, maps, core_ids=list(range(8)))
    return host_output(res.results).astype(np.float32)


# revision 65
# speedup vs baseline: 2.1261x; 1.1526x over previous
"""DeformableConv2D (DCNv2) forward on 8 Trainium2 NeuronCores.

Data-parallel over batch: one sample per core. Per core: offset conv on the
tensor engine (fp16 operands, fp32 accumulate); sampling coordinates and
bilinear weights on the vector engine (sigmoid via odd polynomial, no act
tables); modulated bilinear sampling via SWDGE dma_gather of 2x2-patch rows;
corner combination via packed-fp16 broadcast multiply (DVE 2x mode) +
accumulating PE transposes; im2col GEMM on the tensor engine. The front end
is split into three chunks (4/12/16 blocks) pipelined against the gather
stream, and gather descriptor generation is kept ahead of per-block compute
on the Pool queue.
"""
import sys
sys.path.insert(0, "/opt/trn_rl_repo")

import numpy as np
import ml_dtypes

import concourse.bass as bass
import concourse.bacc as bacc
import concourse.mybir as mybir
import concourse.tile as tile
from concourse import library_config

F32 = mybir.dt.float32
F16 = mybir.dt.float16
I16 = mybir.dt.int16
AL = mybir.AluOpType
AF = mybir.ActivationFunctionType

H = W = 64
C = 128
F = 256
K = 9
PADR = 8                 # padded-coordinate margin
HP = WP = 80             # padded image
NPIX = H * W             # 4096
NBLK = 32                # pixel blocks of 128 (2 rows each)
CONVW = 66               # conv grid width (pad 1)
XCLM = 67 + 9 * 512 + 67  # xcl with shift margins
NROWS = 2 * HP * 40      # pair-table rows = 6400
CHUNKS = [(0, 4), (4, 8), (8, 16), (16, 32)]   # front-end block chunks

# sigmoid(x) ~= 0.5 + x*(C1 + C3 z + C5 z^2 + C7 z^3), z = x^2, |x| <= 2.75
SB = 2.75
SC1, SC3, SC5, SC7 = 0.24955315, -0.019879351, 1.5030454e-3, -5.8584555e-5

DY = np.repeat(np.arange(3) - 1, 3).astype(np.float32)   # per-tap dy
DX = np.tile(np.arange(3) - 1, 3).astype(np.float32)     # per-tap dx


def bcast(ap, shape):
    return ap.to_broadcast(list(shape))


_NC = None


def build_nc():
    nc = bacc.Bacc("TRN2", target_bir_lowering=False,
                   dynamic_dma_scratch_size=32768)
    xcl = nc.dram_tensor("xcl", [C, XCLM], F16, kind="ExternalInput")
    pairs = nc.dram_tensor("pairs", [NROWS, 512], F16, kind="ExternalInput")
    offk = nc.dram_tensor("offk", [C, K * 27], F16, kind="ExternalInput")
    offb = nc.dram_tensor("offb", [1, 27], F16, kind="ExternalInput")
    filt = nc.dram_tensor("filt", [C, K * 2 * 128], F16, kind="ExternalInput")
    eye16 = nc.dram_tensor("eye16", [128, 128], F16, kind="ExternalInput")
    # consts: Y_all [128,32], dy/dx rows [128,9] each, X_all [128,1]
    consts = nc.dram_tensor("consts", [128, 51], F32, kind="ExternalInput")
    out_d = nc.dram_tensor("out", [2, 128, NPIX], F16, kind="ExternalOutput")

    with tile.TileContext(nc) as tc:
        with (
            tc.tile_pool(name="const", bufs=1) as cpool,
            tc.tile_pool(name="front", bufs=1) as fpool,
            tc.tile_pool(name="dram", bufs=1, space="DRAM") as dpool,
            tc.tile_pool(name="convps", bufs=1, space="PSUM") as convpool,
            tc.tile_pool(name="ptps", bufs=1, space="PSUM") as ptpool,
            tc.tile_pool(name="pc0", bufs=2, space="PSUM") as pc0pool,
            tc.tile_pool(name="pc1", bufs=1, space="PSUM") as pc1pool,
            tc.tile_pool(name="pc2", bufs=1, space="PSUM") as pc2pool,
            tc.tile_pool(name="po", bufs=2, space="PSUM") as popool,
            tc.tile_pool(name="sg", bufs=7) as sgpool,
            tc.tile_pool(name="gw", bufs=3) as gwpool,
            tc.tile_pool(name="we8", bufs=8) as we8pool,
            tc.tile_pool(name="blk", bufs=2) as blkpool,
        ):
            nc.gpsimd.load_library(library_config.mlp)

            s_xcl = cpool.tile([C, XCLM], F16)
            nc.sync.dma_start(out=s_xcl[:, 0:800], in_=xcl[:, 0:800])
            s_offk = cpool.tile([C, K * 27], F16)
            nc.sync.dma_start(out=s_offk[:], in_=offk[:])
            s_offb = cpool.tile([1, 27], F16)
            nc.sync.dma_start(out=s_offb[:], in_=offb[:])
            s_const = cpool.tile([128, 51], F32)
            nc.sync.dma_start(out=s_const[:], in_=consts[:])
            s_eye16 = cpool.tile([128, 128], F16)
            nc.sync.dma_start(out=s_eye16[:], in_=eye16[:])
            nc.sync.dma_start(out=s_xcl[:, 800:XCLM], in_=xcl[:, 800:XCLM])
            s_filt = cpool.tile([C, K * 2 * 128], F16)
            nc.sync.dma_start(out=s_filt[:], in_=filt[:])
            s_ones = cpool.tile([1, 264], F16)
            nc.vector.memset(s_ones[:], 1.0)
            s_eye27 = cpool.tile([27, 27], F32)
            nc.vector.tensor_copy(out=s_eye27[:], in_=s_eye16[:27, :27])
            y_all = s_const[:, 0:32]          # [128, 32]
            dy_t = s_const[:, 32:41]          # [128, 9]
            dx_t = s_const[:, 41:50]
            x_all = s_const[:, 50:51]         # [128, 1]

            w16_c = [None] * len(CHUNKS)      # per-chunk weights [128,n,36] f16
            idxw_c = [None] * len(CHUNKS)     # per-chunk wrapped indices
            dst_t = {}                        # per-sg gather destinations

            from contextlib import contextmanager

            @contextmanager
            def prio(base):
                orig = tc.cur_priority
                tc.cur_priority = base
                try:
                    yield
                finally:
                    tc.cur_priority = orig

            def front_chunk(ci):
                """Offset conv + coordinate/weight math for blocks [lo, hi);
                fills w16_c[ci], idxw_c[ci]. High scheduler priority, ordered
                by chunk: this chain feeds the gather stream."""
                with prio(-1000000 + ci * 10000):
                    _front_chunk(ci)

            def _front_chunk(ci):
                lo, hi = CHUNKS[ci]
                n = hi - lo
                wi_c = fpool.tile([27, n, 128], F32, tag=f"wic{ci}")
                wiT = fpool.tile([128, n, 27], F32, tag=f"wiT{ci}")
                for bi in range(0, n, 2):
                    b = lo + bi
                    q0 = (2 * b + 1) * CONVW
                    ps = convpool.tile([27, 264], F32, tag="convps")
                    for t in range(K):
                        d = int(DY[t]) * CONVW + int(DX[t])
                        nc.tensor.matmul(
                            out=ps[:],
                            lhsT=s_offk[:, t * 27:(t + 1) * 27],
                            rhs=s_xcl[:, 67 + q0 + d: 67 + q0 + 264 + d],
                            start=(t == 0), stop=False,
                        )
                    # bias folded in as a rank-1 matmul (offb-row x ones-row)
                    nc.tensor.matmul(out=ps[:], lhsT=s_offb[:], rhs=s_ones[:],
                                     start=False, stop=True)
                    nc.scalar.copy(
                        out=wi_c[:, bi:bi + 2, :].rearrange(
                            "p b (r x) -> p (b r) x", x=64),
                        in_=ps[:].rearrange("p (r x) -> p r x", x=CONVW)[:, :, 1:65])
                    for u in range(2):
                        pt = ptpool.tile([128, 27], F32, tag="pt")
                        nc.tensor.transpose(
                            out=pt[:], in_=wi_c[:, bi + u, :], identity=s_eye27[:])
                        nc.scalar.copy(out=wiT[:, bi + u, :], in_=pt[:])

                # ---- stage B on [128, n, 9] tiles ----
                o1 = wiT[:, :, 0:9]
                o2 = wiT[:, :, 9:18]
                mm = wiT[:, :, 18:27]
                S = [128, n, 9]
                yh = y_all[:, lo:hi]

                def ftile(tag):
                    return fpool.tile(S, F32, tag=f"{tag}{ci}",
                                      name=f"{tag}{ci}")

                # sigmoid via odd polynomial on DVE (no act-table loads)
                xc = ftile("xc")
                nc.vector.tensor_scalar(out=xc[:], in0=mm, scalar1=-SB,
                                        scalar2=SB, op0=AL.max, op1=AL.min)
                zz = ftile("zz")
                nc.vector.tensor_tensor(out=zz[:], in0=xc[:], in1=xc[:],
                                        op=AL.mult)
                sigm = ftile("sigm")
                nc.vector.tensor_scalar(out=sigm[:], in0=zz[:], scalar1=SC7,
                                        scalar2=SC5, op0=AL.mult, op1=AL.add)
                nc.vector.tensor_tensor(out=sigm[:], in0=sigm[:], in1=zz[:],
                                        op=AL.mult)
                nc.vector.tensor_scalar(out=sigm[:], in0=sigm[:], scalar1=SC3,
                                        scalar2=None, op0=AL.add)
                nc.vector.tensor_tensor(out=sigm[:], in0=sigm[:], in1=zz[:],
                                        op=AL.mult)
                nc.vector.tensor_scalar(out=sigm[:], in0=sigm[:], scalar1=SC1,
                                        scalar2=None, op0=AL.add)
                nc.vector.tensor_tensor(out=sigm[:], in0=sigm[:], in1=xc[:],
                                        op=AL.mult)
                nc.vector.tensor_scalar(out=sigm[:], in0=sigm[:], scalar1=0.5,
                                        scalar2=None, op0=AL.add)

                py = ftile("py")
                nc.vector.tensor_tensor(out=py[:], in0=o1, in1=bcast(yh, S),
                                        op=AL.add)
                nc.vector.tensor_tensor(
                    out=py[:], in0=py[:],
                    in1=bcast(dy_t.rearrange("p (o k) -> p o k", o=1), S),
                    op=AL.add)
                nc.vector.tensor_scalar(out=py[:], in0=py[:], scalar1=8.0,
                                        scalar2=2.0, op0=AL.add, op1=AL.max)
                nc.vector.tensor_scalar(out=py[:], in0=py[:], scalar1=77.0,
                                        scalar2=None, op0=AL.min)
                y0p = ftile("y0p")
                nc.vector.tensor_scalar(out=y0p[:], in0=py[:], scalar1=-0.5,
                                        scalar2=8388608.0, op0=AL.add, op1=AL.add)
                nc.vector.tensor_scalar(out=y0p[:], in0=y0p[:], scalar1=-8388608.0,
                                        scalar2=None, op0=AL.add)
                fy = ftile("fy")
                nc.vector.tensor_tensor(out=fy[:], in0=py[:], in1=y0p[:],
                                        op=AL.subtract)
                wy0 = ftile("wy0")
                nc.vector.tensor_scalar(out=wy0[:], in0=fy[:], scalar1=-1.0,
                                        scalar2=1.0, op0=AL.mult, op1=AL.add)

                px = ftile("px")
                nc.vector.tensor_tensor(out=px[:], in0=o2, in1=bcast(x_all, S),
                                        op=AL.add)
                nc.vector.tensor_tensor(
                    out=px[:], in0=px[:],
                    in1=bcast(dx_t.rearrange("p (o k) -> p o k", o=1), S),
                    op=AL.add)
                nc.vector.tensor_scalar(out=px[:], in0=px[:], scalar1=8.0,
                                        scalar2=2.0, op0=AL.add, op1=AL.max)
                nc.vector.tensor_scalar(out=px[:], in0=px[:], scalar1=77.0,
                                        scalar2=None, op0=AL.min)
                x0p = ftile("x0p")
                nc.vector.tensor_scalar(out=x0p[:], in0=px[:], scalar1=-0.5,
                                        scalar2=8388608.0, op0=AL.add, op1=AL.add)
                nc.vector.tensor_scalar(out=x0p[:], in0=x0p[:], scalar1=-8388608.0,
                                        scalar2=None, op0=AL.add)
                fx = ftile("fx")
                nc.vector.tensor_tensor(out=fx[:], in0=px[:], in1=x0p[:],
                                        op=AL.subtract)
                wx0 = ftile("wx0")
                nc.vector.tensor_scalar(out=wx0[:], in0=fx[:], scalar1=-1.0,
                                        scalar2=1.0, op0=AL.mult, op1=AL.add)

                qx = ftile("qx")
                nc.vector.tensor_scalar(out=qx[:], in0=x0p[:], scalar1=0.5,
                                        scalar2=-0.25, op0=AL.mult, op1=AL.add)
                nc.vector.tensor_scalar(out=qx[:], in0=qx[:], scalar1=8388608.0,
                                        scalar2=-8388608.0, op0=AL.add, op1=AL.add)
                parx = ftile("parx")
                nc.vector.scalar_tensor_tensor(
                    out=parx[:], in0=qx[:], scalar=-2.0, in1=x0p[:],
                    op0=AL.mult, op1=AL.add)
                qy = ftile("qy")
                nc.vector.tensor_scalar(out=qy[:], in0=y0p[:], scalar1=0.5,
                                        scalar2=-0.25, op0=AL.mult, op1=AL.add)
                nc.vector.tensor_scalar(out=qy[:], in0=qy[:], scalar1=8388608.0,
                                        scalar2=-8388608.0, op0=AL.add, op1=AL.add)
                pary = ftile("pary")
                nc.vector.scalar_tensor_tensor(
                    out=pary[:], in0=qy[:], scalar=-2.0, in1=y0p[:],
                    op0=AL.mult, op1=AL.add)
                base = ftile("base")
                nc.vector.scalar_tensor_tensor(
                    out=base[:], in0=qy[:], scalar=40.0, in1=qx[:],
                    op0=AL.mult, op1=AL.add)
                nc.vector.scalar_tensor_tensor(
                    out=base[:], in0=parx[:], scalar=1600.0, in1=base[:],
                    op0=AL.mult, op1=AL.add)
                nc.vector.scalar_tensor_tensor(
                    out=base[:], in0=pary[:], scalar=3200.0, in1=base[:],
                    op0=AL.mult, op1=AL.add)

                idx_i16 = fpool.tile([128, n * 9], I16, tag=f"idxi{ci}")
                nc.vector.tensor_copy(
                    out=idx_i16[:].rearrange("p (b k) -> p b k", k=9), in_=base[:])

                # wrapped-index staging: DRAM roundtrip with 2-row-sized
                # descriptors, then an on-chip (pg, g) -> (g, pg) interleave.
                idx_dram = dpool.tile([128, n * 9], I16, tag=f"idxd{ci}")
                nc.sync.dma_start(out=idx_dram[:], in_=idx_i16[:])
                idxw_tmp = fpool.tile([128, n * 72], I16, tag=f"idxt{ci}")
                src = idx_dram[:].rearrange("(pg pp) g -> pp pg g", pg=8)
                for r in range(8):
                    nc.sync.dma_start(
                        out=idxw_tmp[16 * r:16 * (r + 1), :].rearrange(
                            "pp (pg g) -> pp pg g", pg=8),
                        in_=src)
                idxw = fpool.tile([128, n * 72], I16, tag=f"idxw{ci}")
                nc.vector.tensor_copy(
                    out=idxw[:].rearrange("p (g pg) -> p g pg", pg=8),
                    in_=idxw_tmp[:].rearrange("p (pg g) -> p g pg", pg=8))
                idxw_c[ci] = idxw

                # weights W [128, n, 9, 2, 2]  (k, yc, xc)
                a0 = ftile("a0")
                nc.vector.tensor_tensor(out=a0[:], in0=wy0[:], in1=sigm[:],
                                        op=AL.mult)
                a1 = ftile("a1")
                nc.vector.tensor_tensor(out=a1[:], in0=fy[:], in1=sigm[:],
                                        op=AL.mult)
                w_f32 = fpool.tile([128, n, 9, 2, 2], F32, tag=f"wf{ci}")
                nc.vector.tensor_tensor(out=w_f32[:, :, :, 0, 0], in0=a0[:],
                                        in1=wx0[:], op=AL.mult)
                nc.vector.tensor_tensor(out=w_f32[:, :, :, 0, 1], in0=a0[:],
                                        in1=fx[:], op=AL.mult)
                nc.vector.tensor_tensor(out=w_f32[:, :, :, 1, 0], in0=a1[:],
                                        in1=wx0[:], op=AL.mult)
                nc.vector.tensor_tensor(out=w_f32[:, :, :, 1, 1], in0=a1[:],
                                        in1=fx[:], op=AL.mult)
                w16 = fpool.tile([128, n, 36], F16, tag=f"w16{ci}")
                nc.vector.tensor_copy(
                    out=w16[:], in_=w_f32[:].rearrange("p b k y u -> p b (k y u)"))
                w16_c[ci] = w16

            def chunk_of_block(b):
                for ci, (lo, hi) in enumerate(CHUNKS):
                    if lo <= b < hi:
                        return ci, lo
                raise AssertionError

            def emit_gathers(b_lo, b_hi):
                for b in range(b_lo, b_hi):
                    ci, lo = chunk_of_block(b)
                    idxw = idxw_c[ci]
                    g0 = (b - lo) * 9            # slot base within chunk
                    with prio(-900000 + b * 2000):
                        dstb = sgpool.tile([128, K, 512], F16, tag="dst")
                        nc.gpsimd.dma_gather(
                            dstb[:, 0:5, :], pairs[:],
                            idxw[:, g0 * 8:(g0 + 5) * 8],
                            5 * 128, 5 * 128, 512)
                        nc.gpsimd.dma_gather(
                            dstb[:, 5:K, :], pairs[:],
                            idxw[:, (g0 + 5) * 8:(g0 + K) * 8],
                            4 * 128, 4 * 128, 512)
                        dst_t[b] = dstb

            def compute_sg(sg):
                po = [popool.tile([128, 512], F32, tag="po", name=f"po{fc}")
                      for fc in range(2)]
                for bi in range(4):
                    b = 4 * sg + bi
                    ci, lo = chunk_of_block(b)
                    w16 = w16_c[ci]
                    bh = b - lo                  # block within chunk
                    dstb = dst_t.pop(b)
                    w_e8 = we8pool.tile([128, 36, 8], F16, tag="we8")
                    with prio(-890000 + sg * 1000 + bi * 10):
                        nc.scalar.copy(
                            out=w_e8[:],
                            in_=bcast(
                                w16[:, bh, :].rearrange("p (j o) -> p j o", o=1),
                                [128, 36, 8]))
                    gw = gwpool.tile([128, 36, 128], F16, tag="gw")
                    dsrc = dstb[:].rearrange("p s e -> p (s e)").rearrange(
                        "p (j r q) -> p j r q", r=16, q=8)
                    gwv = gw[:].rearrange("p j (r q) -> p j r q", q=8)
                    w_in = bcast(w_e8[:].rearrange("p j (o q) -> p j o q", o=1),
                                 [128, 36, 16, 8])
                    # packed fp16 operands -> DVE 2x mode; prioritized so the
                    # gather destination buffers recycle promptly
                    with prio(-890000 + sg * 1000 + bi * 10):
                        nc.vector.tensor_tensor(out=gwv[:], in0=dsrc[:],
                                                in1=w_in[:], op=AL.mult)
                    cols = blkpool.tile([128, K, 128], F16, tag="colsb")
                    # accumulating transposes: 4 taps per PSUM bank, with the
                    # per-block GEMM chunk interleaved after each bank's copy
                    for kg, pool, nk in ((0, pc0pool, 4), (1, pc1pool, 4),
                                         (2, pc2pool, 1)):
                        pcb = pool.tile([128, nk * 128], F32, tag=f"pc{kg}")
                        for kq in range(nk):
                            k = 4 * kg + kq
                            for j in range(4):
                                nc.tensor.matmul(
                                    out=pcb[:, kq * 128:(kq + 1) * 128],
                                    lhsT=gw[:, 4 * k + j, :],
                                    rhs=s_eye16[:], start=(j == 0), stop=(j == 3))
                        if sg == 7:
                            nc.vector.tensor_copy(
                                out=cols[:, 4 * kg:4 * kg + nk, :],
                                in_=pcb[:].rearrange("p (k c) -> p k c", c=128))
                        else:
                            nc.scalar.copy(
                                out=cols[:, 4 * kg:4 * kg + nk, :],
                                in_=pcb[:].rearrange("p (k c) -> p k c", c=128))
                    for fc in range(2):
                        for k in range(K):
                            nc.tensor.matmul(
                                out=po[fc][:, bi * 128:(bi + 1) * 128],
                                lhsT=s_filt[:, (k * 2 + fc) * 128:
                                            (k * 2 + fc + 1) * 128],
                                rhs=cols[:, k, :],
                                start=(k == 0), stop=(k == K - 1))
                for fc in range(2):
                    osb = blkpool.tile([128, 512], F16, tag="osb")
                    if sg == 7:
                        nc.vector.tensor_copy(out=osb[:], in_=po[fc][:])
                    else:
                        nc.scalar.copy(out=osb[:], in_=po[fc][:])
                    nc.sync.dma_start(
                        out=out_d[fc, :, sg * 512:(sg + 1) * 512], in_=osb[:])

            front_chunk(0)
            emit_gathers(0, 4)
            front_chunk(1)
            emit_gathers(4, 8)
            compute_sg(0)
            front_chunk(2)
            emit_gathers(8, 12)
            compute_sg(1)
            front_chunk(3)
            emit_gathers(12, 16)
            compute_sg(2)
            for sg in range(3, 8):
                if sg + 1 < 8:
                    emit_gathers(4 * sg + 4, 4 * sg + 8)
                compute_sg(sg)
    nc.compile()
    return nc


def host_inputs(x, offset_kernel, offset_bias, filt_w):
    """Per-sample input maps. x [8,64,64,128] f32 etc (numpy)."""
    offk = np.ascontiguousarray(
        offset_kernel.reshape(K, C, 27).transpose(1, 0, 2).reshape(C, K * 27)
    ).astype(np.float16)
    offb = offset_bias.reshape(1, 27).astype(np.float16)
    filt_re = np.ascontiguousarray(
        filt_w.reshape(K, C, 2, 128).transpose(1, 0, 2, 3).reshape(C, K * 2 * 128)
    ).astype(np.float16)
    eye16 = np.eye(128).astype(np.float16)
    consts = np.zeros((128, 51), np.float32)
    p = np.arange(128)
    yoff = p // 64
    consts[:, 0:32] = 2 * np.arange(32)[None, :] + yoff[:, None]
    consts[:, 32:41] = DY[None, :]
    consts[:, 41:50] = DX[None, :]
    consts[:, 50] = p % 64

    maps = []
    for b in range(x.shape[0]):
        xp = np.zeros((HP + 2, WP + 2, C), np.float32)
        xp[PADR:PADR + H, PADR:PADR + W] = x[b]
        quad = np.zeros((2, 2, 40, 40, 2, 2, C), np.float32)
        for pY in range(2):
            for pX in range(2):
                for uy in range(2):
                    for ux in range(2):
                        quad[pY, pX, :, :, uy, ux] = \
                            xp[pY + uy:pY + uy + 80:2, pX + ux:pX + ux + 80:2]
        prs = quad.reshape(NROWS, 4 * C).astype(np.float16)

        x1 = np.zeros((CONVW, CONVW, C), np.float32)
        x1[1:65, 1:65] = x[b]
        xcl = np.zeros((C, XCLM), np.float16)
        xcl[:, 67:67 + 4356] = x1.reshape(CONVW * CONVW, C).T.astype(np.float16)
        maps.append({
            "xcl": xcl, "pairs": prs, "offk": offk, "offb": offb,
            "filt": filt_re, "eye16": eye16, "consts": consts,
        })
    return maps


def host_output(res_list):
    outs = []
    for r in res_list:
        o = r["out"].astype(np.float32).reshape(256, NPIX)
        outs.append(np.ascontiguousarray(o.T).reshape(H, W, F))
    return np.stack(outs)


def _get_nc():
    global _NC
    if _NC is None:
        _NC = build_nc()
    return _NC


def kernel(inputs, offset_kernel, offset_bias, filt):
    from concourse.bass_utils import run_bass_kernel_spmd
    x = np.asarray(inputs, dtype=np.float32)
    maps = host_inputs(x, np.asarray(offset_kernel, np.float32),
                       np.asarray(offset_bias, np.float32),
                       np.asarray(filt, np.float32))
    nc = _get_nc()
    res = run_bass_kernel_spmd(nc, maps, core_ids=list(range(8)))
    return host_output(res.results).astype(np.float32)
